# revision 34
# baseline (speedup 1.0000x reference)
"""Trainium2 Bass kernel for nn_MILPAttention (dense multi-head attention with
per-key additive bias), tensor-parallel over heads across 8 NeuronCores.

Self-contained: hardcodes shapes N=4096, D=1024, H=16, GAMMA=1.0.

Math (reference):
    q = x @ Wq.T + bq ; k = x @ Wk.T + bk ; v = x @ Wv.T + bv     (per head, dh=64)
    logits = (q @ k.T) / 8 - h[key]
    attn = softmax(logits, keys)
    out = (attn @ v) @ Wo.T + bo + x

Per-core strategy (core i owns heads 2i, 2i+1 = columns 128i:128(i+1)):
  - Projections run as fp8 DoubleRow matmuls over resident x^T, evacuated
    at N=1024. q is computed into two half-zeroed tiles qa (head0 in rows
    0:64) / qb (head1 in rows 64:128) so every S matmul runs with a full
    K=128 contraction; q is pre-scaled by m8 = 8/ln2 (Schraudolph prep).
  - S^T per key chunk: head0 into cols 0:512, head1 into 512:1024 of one
    [128,1024] PSUM tile, consumed WHOLE by one exp engine at N=1024.
    Scalar tiles: true exp (ACT Exp -> fp8e4). Vector tiles: Schraudolph
    tensor_scalar add+max -> int8 bits = fp8e4 of exp(l-c). An 18/14 kc
    split balances the engines; ScalarE+VectorE PSUM-evacuation throughput
    and the PE (at its ~2.0 GHz sustained clock) are jointly the roofline.
    The global shift c cancels in the softmax ratio.
  - P@V as fp8 DoubleRow matmuls, deferred two pairs so the in-order PE
    queue never waits on the exp engines. V transposed via PE, scaled by
    w = exp(-h) (folds the per-key bias), stored fp8 with w as a 65th
    stationary column so PV also yields the softmax denominator.
  - Normalize reads the PV psum directly (no snap copy): DVE reciprocal of
    the denominator row, gpsimd partition-broadcast, DVE multiply -> fp8.
  - 4-way split AllToAll (fp8) switches head-sharding -> sequence-sharding
    (quarters issued at q-blocks 2/4/6/7; ~16us latency each, only the
    last is exposed); out projection fp8 DoubleRow per quarter, bias +
    residual in fp32.
"""
import numpy as np

import concourse.bass as bass
import concourse.mybir as mybir
import concourse.tile as tile
from concourse import bacc
from concourse.bass_utils import run_bass_kernel_spmd
from concourse.masks import make_identity

N, D, H = 4096, 1024, 16
NCORE = 8
CB = D // NCORE          # 128 columns (2 heads) per core
NR = N // NCORE          # 512 output rows per core
DH = D // H              # 64
KCH = N // 128           # 32 key chunks
NB = N // 512            # 8 n-blocks
FP = mybir.dt.float32
BF = mybir.dt.bfloat16
F8 = mybir.dt.float8e4
I8 = mybir.dt.int8
AF = mybir.ActivationFunctionType
ALU = mybir.AluOpType
DR = mybir.MatmulPerfMode.DoubleRow

M8 = 8.0 / np.log(2.0)      # 11.5416 Schraudolph scale, folded into Wq on host
CSHIFT = 4.8                # global logit shift: P' = exp(l - c), cancels in softmax
CORR = -0.47                # Schraudolph correction (round-to-nearest tuned)
BPRIME = 56.0 + CORR - M8 * CSHIFT   # add constant (psum already holds m8*l)

NQ = 4                       # A2A quarters, 2 phys q-blocks (128 rows) each


def _body(nc, tc, reps, xt, xr, wqt, wkt, wvt, wot, bqv, bkv, bvv, bov, hv, out,
          use_collective=True, dbg=None):
    cst = tc.alloc_tile_pool(name="cst", bufs=1)
    per = tc.alloc_tile_pool(name="per", bufs=1)
    dram = tc.alloc_tile_pool(name="dram", bufs=1, space="DRAM")

    ident = cst.tile([128, 128], BF)
    make_identity(nc, ident[:])

    # persistent sbuf
    wq_b = per.tile([128, D], F8)        # [d-in-chunk, dc*128 + c]
    wk_b = per.tile([128, D], F8)
    wv_b = per.tile([128, D], F8)
    wo_b = per.tile([128, 8 * D], F8)    # [c-in-chunk, cc*1024 + o]
    qa_t = per.tile([128, N], BF)        # head0 q in rows 0:64, zeros 64:128
    qb_t = per.tile([128, N], BF)        # head1 q in rows 64:128, zeros 0:64
    kb_t = per.tile([128, N], BF)        # kT: rows = 2 heads x 64 dims
    vw8 = per.tile([128, KCH * 256], F8)  # per kc, per head: [w|pad63|v64]
    bq_s = per.tile([128, 1], FP)
    bk_s = per.tile([128, 1], FP)
    bv_s = per.tile([128, 1], FP)
    cbias = per.tile([128, 1], FP)       # -CSHIFT for the scalar exp
    cscale = per.tile([128, 1], FP)      # 1/M8 for the scalar exp
    w_s = per.tile([128, KCH], FP)       # exp(-h), [key-in-chunk, chunk]
    xb_s = [per.tile([128, D], FP, name=f"xb{j}") for j in range(4)]  # x rows + bo
    x_all = per.tile([128, 8 * N], F8)   # x^T resident, col = dc*N + n

    cc_in = [dram.tile([NCORE * 128, 128], F8, name=f"ccin{i}") for i in range(NQ)]
    cc_out = [dram.tile([NCORE * 128, 128], F8, name=f"ccout{i}") for i in range(NQ)]

    def a2a(qq):
        nc.gpsimd.collective_compute(
            "AllToAll", mybir.AluOpType.bypass,
            replica_groups=[list(range(NCORE))],
            ins=[cc_in[qq][:].opt()], outs=[cc_out[qq][:].opt()])

    for rep in range(reps):
        sfx = f"_{rep}"
        # ---------------- phase 0: constants ------------------------------
        hst = per.tile([128, KCH], FP, name="hst")
        nc.sync.dma_start(hst[:], hv)
        nc.scalar.activation(w_s[:], hst[:], AF.Exp, scale=-1.0)
        nc.sync.dma_start(bq_s[:], bqv.unsqueeze(1))
        nc.scalar.dma_start(bk_s[:], bkv.unsqueeze(1))
        nc.scalar.dma_start(bv_s[:], bvv.unsqueeze(1))
        nc.gpsimd.memset(cbias[:], -CSHIFT)
        nc.gpsimd.memset(cscale[:], 1.0 / M8)
        nc.vector.memset(qa_t[64:128, :], 0.0)
        nc.gpsimd.memset(qb_t[0:64, :], 0.0)
        for wi, (wsrc, wdst) in enumerate(
                ((wqt, wq_b), (wkt, wk_b), (wvt, wv_b))):
            eng = (nc.sync, nc.scalar, nc.scalar)[wi]
            eng.dma_start(wdst[:].rearrange("p (dc c) -> p dc c", c=CB),
                          wsrc.rearrange("(dc p) c -> p dc c", p=128))
        # w columns of vw8 (first col of each head block) <- w_s[:, kc]
        for h in range(2):
            nc.vector.tensor_copy(
                vw8[:].rearrange("p (kc c) -> p kc c", c=256)[:, :, 128 * h],
                w_s[:])

        # ------ phases 1+2 share one scope (no cross-phase PE barrier) -----
        vw8_r = vw8[:].rearrange("p (pr two c) -> p pr two c", two=2, c=256)
        with tc.tile_pool(name="p1s" + sfx, bufs=4) as p1s, \
             tc.tile_pool(name="p2s" + sfx, bufs=5) as p2s, \
             tc.tile_pool(name="aop" + sfx, bufs=4) as aop, \
             tc.tile_pool(name="p2n", bufs=2) as p2n, \
             tc.tile_pool(name="psS", bufs=3, space="PSUM") as psS, \
             tc.tile_pool(name="psO", bufs=1, space="PSUM") as psO:
            # ---------- phase 1: x preload + projections (N=1024 blocks) ---
            def load_pair(nbp):
                for nb in (2 * nbp, 2 * nbp + 1):
                    for dc in range(8):
                        (nc.sync, nc.scalar, nc.gpsimd)[dc % 3].dma_start(
                            x_all[:, dc * N + nb * 512:dc * N + (nb + 1) * 512],
                            xt[dc * 128:(dc + 1) * 128, nb * 512:(nb + 1) * 512])

            def proj_mm(ps, w_b, ncol1024):
                # DR moving operand caps at 2x512 elements: two 512-col groups
                for g in range(2):
                    ncol = slice(ncol1024.start + g * 512,
                                 ncol1024.start + (g + 1) * 512)
                    for dp in range(4):
                        nc.tensor.matmul(
                            ps[:, g * 512:(g + 1) * 512],
                            w_b[:].rearrange("p (dp two c) -> p dp two c",
                                             two=2, c=CB)[:, dp],
                            x_all[:].rearrange("p (dp two n) -> p dp two n",
                                               two=2, n=N)[:, dp, :, ncol],
                            start=(dp == 0), stop=(dp == 3), perf_mode=DR)

            load_pair(0)
            for nbp in range(4):
                ncol = slice(nbp * 1024, (nbp + 1) * 1024)
                psk = psS.tile([128, 1024], FP, name="pst")
                proj_mm(psk, wk_b, ncol)
                nc.scalar.activation(kb_t[:, ncol], psk[:], AF.Identity,
                                     bias=bk_s[:, 0:1])
                psv = psS.tile([128, 1024], FP, name="pst")
                proj_mm(psv, wv_b, ncol)
                vtb = p1s.tile([128, 1024], BF, name="vtb")
                nc.vector.tensor_scalar_add(vtb[:], psv[:], bv_s[:, 0:1])
                pvt_f = psS.tile([128, 1024], FP, name="pst")
                pvt = pvt_f[:].bitcast(BF)  # [128, 2048] bf16 view
                for half in range(2):  # nb = 2*nbp + half
                    for ns in range(4):
                        c = half * 512 + ns * 128
                        nc.tensor.transpose(pvt[:, c:c + 128],
                                            vtb[:, c:c + 128], ident[:])
                    for ns in range(4):
                        kc = (2 * nbp + half) * 4 + ns
                        c0 = kc * 256
                        c = half * 512 + ns * 128
                        nc.vector.tensor_scalar_mul(
                            vw8[:, c0 + 64:c0 + 128],
                            pvt[:, c:c + 64], w_s[:, kc:kc + 1])
                        nc.vector.tensor_scalar_mul(
                            vw8[:, c0 + 192:c0 + 256],
                            pvt[:, c + 64:c + 128], w_s[:, kc:kc + 1])
                psq = psS.tile([128, 1024], FP, name="pst")
                proj_mm(psq, wq_b, ncol)
                # striped query layout: phys col s*512 + nb*64 + t holds
                # global query nb*512 + s*64 + t
                for ht, qdst in ((0, qa_t), (1, qb_t)):
                    rows = slice(64 * ht, 64 * ht + 64)
                    q_dst = qdst[rows, :].rearrange(
                        "p (s j t) -> p s j t", j=8, t=64)[:, :, 2 * nbp:2 * nbp + 2, :]
                    nc.scalar.activation(
                        q_dst,
                        psq[rows, :].rearrange("p (j s t) -> p s j t", j=2, t=64),
                        AF.Identity, bias=bq_s[rows, 0:1])
                if nbp < 3:  # prefetch next x pair (keeps first-MM deps tight)
                    load_pair(nbp + 1)

            # ------- prefetch: out-proj consts (DMA-idle window) -----------
            nc.sync.dma_start(wo_b[:].rearrange("p (cc o) -> p cc o", o=D),
                              wot.rearrange("(cc p) o -> p cc o", p=128))
            bost = p1s.tile([128, D], FP, name="bost")
            nc.sync.dma_start(bost[:], bov.unsqueeze(0).broadcast_to([128, D]))
            for j in range(4):
                xrt = p1s.tile([128, D], FP, name=f"xrt{j % 2}")
                (nc.sync if j % 2 == 0 else nc.scalar).dma_start(
                    xrt[:], xr[j * 128:(j + 1) * 128, :])
                nc.gpsimd.tensor_tensor(xb_s[j][:], xrt[:], bost[:], op=ALU.add)

            # ---------- phase 2: attention loop ----------------------------
            pb_q = []  # deferred PV inputs: (pso set, q5, pr, pb8 tile)

            def normalize(psoX, q5_):
                # ao = pso[64:128] * recip(pso[0]) -> fp8, then stage for A2A
                for h in range(2):
                    rc = p2n.tile([1, 512], FP, name=f"rc{h}")
                    nc.vector.reciprocal_approx_fast(rc[:], psoX[h][0:1, :])
                    bch = p2n.tile([128, 512], FP, name=f"bc{h}")
                    nc.gpsimd.partition_broadcast(bch[:], rc[:])
                    aot = aop.tile([128, 512], F8, name=f"ao{h}")
                    nc.vector.tensor_mul(aot[64:128, :], psoX[h][64:128, :],
                                         bch[64:128, :])
                    # stage phys block q5_ (stripe q5_ of every dest)
                    qq, bc = q5_ // 2, q5_ % 2
                    nc.sync.dma_start(
                        cc_in[qq][:].rearrange("(j g) c -> g j c", g=128)
                        [h * 64:(h + 1) * 64, :, bc * 64:(bc + 1) * 64],
                        aot[64:128, :].rearrange("p (j t) -> p j t", t=64))
                    if dbg is not None:
                        nc.scalar.dma_start(
                            dbg["d_ao"][h * 64:(h + 1) * 64,
                                        q5_ * 512:(q5_ + 1) * 512],
                            aot[64:128, :])
                if use_collective and q5_ in (1, 3, 5):
                    a2a(q5_ // 2)  # both blocks of the quarter now staged


            def emit_pv():
                psoX, q5_, pr_, pbt_ = pb_q.pop(0)
                pbr_ = pbt_[:].rearrange("p (kc h c) -> p kc h c", kc=2, h=2)
                for h in range(2):
                    nc.tensor.matmul(
                        psoX[h][:], vw8_r[:, pr_, :, 128 * h:128 * h + 128],
                        pbr_[:, :, h],
                        start=(pr_ == 0), stop=(pr_ == 15), perf_mode=DR)
                if pr_ == 15:  # q5_ complete: normalize while q5_+1 computes
                    normalize(psoX, q5_)

            for q5 in range(N // 512):
                qcol = slice(q5 * 512, (q5 + 1) * 512)
                pso = [psO.tile([128, 512], FP, name=f"pso{h}") for h in range(2)]
                for pr in range(16):
                    pb8 = p2s.tile([128, 2048], F8, name="pb8")
                    for i in range(2):
                        kc = 2 * pr + i
                        krng = slice(kc * 128, (kc + 1) * 128)
                        # one [128,1024] psum tile per kc: head0 S in cols
                        # 0:512, head1 in 512:1024 (zero-padded K=128)
                        pst = psS.tile([128, 1024], FP, name="pst")
                        nc.tensor.matmul(pst[:, 0:512], kb_t[:, krng],
                                         qa_t[:, qcol], start=True, stop=True)
                        nc.tensor.matmul(pst[:, 512:1024], kb_t[:, krng],
                                         qb_t[:, qcol], start=True, stop=True)
                        # whole-tile exp, strictly alternating engines so no
                        # two consecutive tiles hit the same engine (stalls)
                        if kc % 2 == 1:
                            dst = pb8[:].bitcast(I8)[:, i * 1024:(i + 1) * 1024]
                            nc.vector.tensor_scalar(dst, pst[:], BPRIME, 0.0,
                                                    op0=ALU.add, op1=ALU.max)
                        else:
                            nc.scalar.activation(
                                pb8[:, i * 1024:(i + 1) * 1024], pst[:],
                                AF.Exp, bias=cbias[:, 0:1], scale=cscale[:, 0:1])
                    pb_q.append((pso, q5, pr, pb8))
                    if len(pb_q) > 2:
                        emit_pv()  # deferred 2 pairs; crosses q5 boundaries
            while pb_q:
                emit_pv()  # prs 14, 15 of the last block (+ its normalize)

        # ---------------- phase 3: last A2A + out projection ---------------
        with tc.tile_pool(name="p3s" + sfx, bufs=16) as p3s, \
             tc.tile_pool(name="p3f" + sfx, bufs=4) as p3f, \
             tc.tile_pool(name="p3p", bufs=1, space="PSUM") as p3p:
            if use_collective:
                a2a(3)
            else:  # single-core timing-sim stand-in
                for i in range(NQ):
                    nc.sync.dma_start(cc_out[i][:], cc_in[i][:])
            # quarter qq = my output rows qq*128:(qq+1)*128 (after striping)
            psf = [p3p.tile([128, 512], FP, name=f"psf{t}") for t in range(8)]
            wo_r = wo_b[:].rearrange("p (pr two o) -> p pr two o", two=2, o=D)
            for qq in range(NQ):
                aocs = []
                for cp in range(4):
                    aoc = p3s.tile([128, 256], F8, name="aoc")
                    (nc.sync, nc.scalar, nc.gpsimd)[cp % 3].dma_start(
                        aoc[:].rearrange("p (two q) -> p two q", two=2),
                        cc_out[qq][cp * 256:(cp + 1) * 256, :].rearrange(
                            "(two p) q -> p two q", p=128))
                    aocs.append(aoc)
                for cp in range(4):
                    aoc_r = aocs[cp][:].rearrange("p (two q) -> p two q", two=2)
                    for ob in range(2):
                        nc.tensor.matmul(
                            psf[qq * 2 + ob][:], aoc_r,
                            wo_r[:, cp, :, ob * 512:(ob + 1) * 512],
                            start=(cp == 0), stop=(cp == 3), perf_mode=DR)
                for ob in range(2):
                    t = qq * 2 + ob
                    fo = p3f.tile([128, 512], FP, name="fo")
                    nc.vector.tensor_add(fo[:], psf[t][:],
                                         xb_s[qq][:, ob * 512:(ob + 1) * 512])
                    (nc.sync, nc.scalar, nc.gpsimd)[t % 3].dma_start(
                        out[qq * 128:(qq + 1) * 128,
                            ob * 512:(ob + 1) * 512], fo[:])

    if dbg is not None:
        for nm, t in (("d_q", qa_t), ("d_k", kb_t), ("d_vw", vw8)):
            nc.sync.dma_start(dbg[nm], t[:])
        for i in range(NQ):
            nc.sync.dma_start(dbg["d_cc"][:, i * 128:(i + 1) * 128], cc_out[i][:])
    dram.release()
    per.release()
    cst.release()


def build_nc(reps=1, use_collective=True, debug=False):
    nc = bacc.Bacc("TRN2", target_bir_lowering=False, debug=False, num_devices=NCORE)
    xt = nc.dram_tensor("xt", [D, N], F8, kind="ExternalInput").ap()
    xr = nc.dram_tensor("xr", [NR, D], FP, kind="ExternalInput").ap()
    wqt = nc.dram_tensor("wqt", [D, CB], F8, kind="ExternalInput").ap()
    wkt = nc.dram_tensor("wkt", [D, CB], F8, kind="ExternalInput").ap()
    wvt = nc.dram_tensor("wvt", [D, CB], F8, kind="ExternalInput").ap()
    wot = nc.dram_tensor("wot", [D, D], F8, kind="ExternalInput").ap()
    bqv = nc.dram_tensor("bqv", [CB], FP, kind="ExternalInput").ap()
    bkv = nc.dram_tensor("bkv", [CB], FP, kind="ExternalInput").ap()
    bvv = nc.dram_tensor("bvv", [CB], FP, kind="ExternalInput").ap()
    bov = nc.dram_tensor("bov", [D], FP, kind="ExternalInput").ap()
    # h pre-rearranged on host to [128, KCH]: hv[p, c] = h[c*128 + p]
    hv = nc.dram_tensor("hv", [128, KCH], FP, kind="ExternalInput").ap()
    out = nc.dram_tensor("out", [NR, D], FP, kind="ExternalOutput").ap()
    dbg = None
    if debug:
        dbg = {
            "d_q": nc.dram_tensor("d_q", [128, N], BF, kind="ExternalOutput").ap(),
            "d_k": nc.dram_tensor("d_k", [128, N], BF, kind="ExternalOutput").ap(),
            "d_vw": nc.dram_tensor("d_vw", [128, KCH * 256], F8, kind="ExternalOutput").ap(),
            "d_ao": nc.dram_tensor("d_ao", [128, N], F8, kind="ExternalOutput").ap(),
            "d_cc": nc.dram_tensor("d_cc", [NCORE * 128, NR], F8, kind="ExternalOutput").ap(),
        }
    with tile.TileContext(nc) as tc:
        _body(nc, tc, reps, xt, xr, wqt, wkt, wvt, wot,
              bqv, bkv, bvv, bov, hv, out, use_collective=use_collective, dbg=dbg)
    nc.compile()
    return nc


_NC_CACHE = {}


def get_nc(reps=1):
    if reps not in _NC_CACHE:
        _NC_CACHE[reps] = build_nc(reps)
    return _NC_CACHE[reps]


def make_in_maps(inputs):
    x = np.ascontiguousarray(np.asarray(inputs["x"], dtype=np.float32))
    h = np.ascontiguousarray(np.asarray(inputs["h"], dtype=np.float32))
    Wq = np.asarray(inputs["Wq"], dtype=np.float32)
    bq = np.asarray(inputs["bq"], dtype=np.float32)
    Wk = np.asarray(inputs["Wk"], dtype=np.float32)
    bk = np.asarray(inputs["bk"], dtype=np.float32)
    Wv = np.asarray(inputs["Wv"], dtype=np.float32)
    bv = np.asarray(inputs["bv"], dtype=np.float32)
    Wo = np.asarray(inputs["Wo"], dtype=np.float32)
    bo = np.ascontiguousarray(np.asarray(inputs["bo"], dtype=np.float32))
    import ml_dtypes
    f8 = ml_dtypes.float8_e4m3
    xt = np.ascontiguousarray(x.T.astype(f8))
    wot = np.ascontiguousarray(Wo.T.astype(f8))
    qscale = np.float32(0.125 * M8)  # 1/sqrt(dh) * Schraudolph scale
    in_maps = []
    for i in range(NCORE):
        cs = slice(i * CB, (i + 1) * CB)
        in_maps.append({
            "xt": xt,
            "xr": np.ascontiguousarray(x[i * NR:(i + 1) * NR, :]),
            "wqt": np.ascontiguousarray((Wq[cs, :] * qscale).T.astype(f8)),
            "wkt": np.ascontiguousarray(Wk[cs, :].T.astype(f8)),
            "wvt": np.ascontiguousarray(Wv[cs, :].T.astype(f8)),
            "wot": wot,
            "bqv": np.ascontiguousarray(bq[cs] * qscale),
            "bkv": np.ascontiguousarray(bk[cs]),
            "bvv": np.ascontiguousarray(bv[cs]),
            "bov": bo,
            "hv": np.ascontiguousarray(h.reshape(KCH, 128).T),
        })
    return in_maps


def kernel(**inputs):
    nc = get_nc(1)
    in_maps = make_in_maps(inputs)
    res = run_bass_kernel_spmd(nc, in_maps, core_ids=list(range(NCORE)))
    return np.concatenate([res.results[i]["out"] for i in range(NCORE)], axis=0)


# revision 40
# speedup vs baseline: 1.0146x; 1.0146x over previous
"""Trainium2 Bass kernel for nn_MILPAttention (dense multi-head attention with
per-key additive bias), tensor-parallel over heads across 8 NeuronCores.

Self-contained: hardcodes shapes N=4096, D=1024, H=16, GAMMA=1.0.

Math (reference):
    q = x @ Wq.T + bq ; k = x @ Wk.T + bk ; v = x @ Wv.T + bv     (per head, dh=64)
    logits = (q @ k.T) / 8 - h[key]
    attn = softmax(logits, keys)
    out = (attn @ v) @ Wo.T + bo + x

Per-core strategy (core i owns heads 2i, 2i+1 = columns 128i:128(i+1)):
  - Projections run as fp8 DoubleRow matmuls over resident x^T, evacuated
    at N=1024. q is computed into two half-zeroed tiles qa (head0 in rows
    0:64) / qb (head1 in rows 64:128) so every S matmul runs with a full
    K=128 contraction; q is pre-scaled by m8 = 8/ln2 (Schraudolph prep).
  - S^T per key chunk: head0 into cols 0:512, head1 into 512:1024 of one
    [128,1024] PSUM tile, consumed WHOLE by one exp engine at N=1024.
    Scalar tiles: true exp (ACT Exp -> fp8e4). Vector tiles: Schraudolph
    tensor_scalar add+max -> int8 bits = fp8e4 of exp(l-c). An 18/14 kc
    split balances the engines; ScalarE+VectorE PSUM-evacuation throughput
    and the PE (at its ~2.0 GHz sustained clock) are jointly the roofline.
    The global shift c cancels in the softmax ratio.
  - P@V as fp8 DoubleRow matmuls, deferred two pairs so the in-order PE
    queue never waits on the exp engines. V transposed via PE, scaled by
    w = exp(-h) (folds the per-key bias), stored fp8 with w as a 65th
    stationary column so PV also yields the softmax denominator.
  - Normalize reads the PV psum directly (no snap copy): DVE reciprocal of
    the denominator row, gpsimd partition-broadcast, DVE multiply -> fp8.
  - 4-way split AllToAll (fp8) switches head-sharding -> sequence-sharding
    (quarters issued at q-blocks 2/4/6/7; ~16us latency each, only the
    last is exposed); out projection fp8 DoubleRow per quarter, bias +
    residual in fp32.
"""
import numpy as np

import concourse.bass as bass
import concourse.mybir as mybir
import concourse.tile as tile
from concourse import bacc
from concourse.bass_utils import run_bass_kernel_spmd
from concourse.masks import make_identity

N, D, H = 4096, 1024, 16
NCORE = 8
CB = D // NCORE          # 128 columns (2 heads) per core
NR = N // NCORE          # 512 output rows per core
DH = D // H              # 64
KCH = N // 128           # 32 key chunks
NB = N // 512            # 8 n-blocks
FP = mybir.dt.float32
BF = mybir.dt.bfloat16
F8 = mybir.dt.float8e4
I8 = mybir.dt.int8
AF = mybir.ActivationFunctionType
ALU = mybir.AluOpType
DR = mybir.MatmulPerfMode.DoubleRow

M8 = 8.0 / np.log(2.0)      # 11.5416 Schraudolph scale, folded into Wq on host
CSHIFT = 4.8                # global logit shift: P' = exp(l - c), cancels in softmax
CORR = -0.47                # Schraudolph correction (round-to-nearest tuned)
BPRIME = 56.0 + CORR - M8 * CSHIFT   # add constant (psum already holds m8*l)

NQ = 4                       # A2A quarters, 2 phys q-blocks (128 rows) each


def _body(nc, tc, reps, xt, xr, wqt, wkt, wvt, wot, bqv, bkv, bvv, bov, hv, out,
          use_collective=True, dbg=None):
    cst = tc.alloc_tile_pool(name="cst", bufs=1)
    per = tc.alloc_tile_pool(name="per", bufs=1)
    dram = tc.alloc_tile_pool(name="dram", bufs=1, space="DRAM")

    ident = cst.tile([128, 128], BF)
    make_identity(nc, ident[:])

    # persistent sbuf
    wq_b = per.tile([128, D], F8)        # [d-in-chunk, dc*128 + c]
    wk_b = per.tile([128, D], F8)
    wv_b = per.tile([128, D], F8)
    wo_b = per.tile([128, 8 * D], F8)    # [c-in-chunk, cc*1024 + o]
    # concatenated q: cols 0:N = head0 q (rows 0:64 real, 64:128 zero),
    # cols N:2N = head1 q (rows 64:128 real, 0:64 zero). One [128,2,512]
    # strided moving AP then computes BOTH heads' S in a single matmul.
    qab_t = per.tile([128, 2 * N], BF)
    kb_t = per.tile([128, N], BF)        # kT: rows = 2 heads x 64 dims
    vw8 = per.tile([128, KCH * 256], F8)  # per kc, per head: [w|pad63|v64]
    bq_s = per.tile([128, 1], FP)
    bk_s = per.tile([128, 1], FP)
    bv_s = per.tile([128, 1], FP)
    cbias = per.tile([128, 1], FP)       # -CSHIFT for the scalar exp
    cscale = per.tile([128, 1], FP)      # 1/M8 for the scalar exp
    w_s = per.tile([128, KCH], FP)       # exp(-h), [key-in-chunk, chunk]
    xb_s = [per.tile([128, D], FP, name=f"xb{j}") for j in range(4)]  # x rows + bo
    x_all = per.tile([128, 8 * N], F8)   # x^T resident, col = dc*N + n

    cc_in = [dram.tile([NCORE * 128, 128], F8, name=f"ccin{i}") for i in range(NQ)]
    cc_out = [dram.tile([NCORE * 128, 128], F8, name=f"ccout{i}") for i in range(NQ)]

    def a2a(qq):
        nc.gpsimd.collective_compute(
            "AllToAll", mybir.AluOpType.bypass,
            replica_groups=[list(range(NCORE))],
            ins=[cc_in[qq][:].opt()], outs=[cc_out[qq][:].opt()])

    for rep in range(reps):
        sfx = f"_{rep}"
        # ---------------- phase 0: constants ------------------------------
        hst = per.tile([128, KCH], FP, name="hst")
        nc.sync.dma_start(hst[:], hv)
        nc.scalar.activation(w_s[:], hst[:], AF.Exp, scale=-1.0)
        nc.sync.dma_start(bq_s[:], bqv.unsqueeze(1))
        nc.scalar.dma_start(bk_s[:], bkv.unsqueeze(1))
        nc.scalar.dma_start(bv_s[:], bvv.unsqueeze(1))
        nc.gpsimd.memset(cbias[:], -CSHIFT)
        nc.gpsimd.memset(cscale[:], 1.0 / M8)
        nc.vector.memset(qab_t[64:128, 0:N], 0.0)
        nc.gpsimd.memset(qab_t[0:64, N:2 * N], 0.0)
        for wi, (wsrc, wdst) in enumerate(
                ((wqt, wq_b), (wkt, wk_b), (wvt, wv_b))):
            eng = (nc.sync, nc.scalar, nc.scalar)[wi]
            eng.dma_start(wdst[:].rearrange("p (dc c) -> p dc c", c=CB),
                          wsrc.rearrange("(dc p) c -> p dc c", p=128))
        # w columns of vw8 (first col of each head block) <- w_s[:, kc]
        for h in range(2):
            nc.vector.tensor_copy(
                vw8[:].rearrange("p (kc c) -> p kc c", c=256)[:, :, 128 * h],
                w_s[:])

        # ------ phases 1+2 share one scope (no cross-phase PE barrier) -----
        vw8_r = vw8[:].rearrange("p (pr two c) -> p pr two c", two=2, c=256)
        with tc.tile_pool(name="p1s" + sfx, bufs=4) as p1s, \
             tc.tile_pool(name="p2s" + sfx, bufs=5) as p2s, \
             tc.tile_pool(name="aop" + sfx, bufs=4) as aop, \
             tc.tile_pool(name="p2n", bufs=2) as p2n, \
             tc.tile_pool(name="psS", bufs=3, space="PSUM") as psS, \
             tc.tile_pool(name="psO", bufs=1, space="PSUM") as psO:
            # ---------- phase 1: x preload + projections (N=1024 blocks) ---
            def load_pair(nbp):
                for nb in (2 * nbp, 2 * nbp + 1):
                    for dc in range(8):
                        (nc.sync, nc.scalar, nc.gpsimd)[dc % 3].dma_start(
                            x_all[:, dc * N + nb * 512:dc * N + (nb + 1) * 512],
                            xt[dc * 128:(dc + 1) * 128, nb * 512:(nb + 1) * 512])

            def proj_mm(ps, w_b, ncol1024):
                # DR moving operand caps at 2x512 elements: two 512-col groups
                for g in range(2):
                    ncol = slice(ncol1024.start + g * 512,
                                 ncol1024.start + (g + 1) * 512)
                    for dp in range(4):
                        nc.tensor.matmul(
                            ps[:, g * 512:(g + 1) * 512],
                            w_b[:].rearrange("p (dp two c) -> p dp two c",
                                             two=2, c=CB)[:, dp],
                            x_all[:].rearrange("p (dp two n) -> p dp two n",
                                               two=2, n=N)[:, dp, :, ncol],
                            start=(dp == 0), stop=(dp == 3), perf_mode=DR)

            load_pair(0)
            for nbp in range(4):
                ncol = slice(nbp * 1024, (nbp + 1) * 1024)
                psk = psS.tile([128, 1024], FP, name="pst")
                proj_mm(psk, wk_b, ncol)
                nc.scalar.activation(kb_t[:, ncol], psk[:], AF.Identity,
                                     bias=bk_s[:, 0:1])
                psv = psS.tile([128, 1024], FP, name="pst")
                proj_mm(psv, wv_b, ncol)
                vtb = p1s.tile([128, 1024], BF, name="vtb")
                nc.vector.tensor_scalar_add(vtb[:], psv[:], bv_s[:, 0:1])
                pvt_f = psS.tile([128, 1024], FP, name="pst")
                pvt = pvt_f[:].bitcast(BF)  # [128, 2048] bf16 view
                for half in range(2):  # nb = 2*nbp + half
                    for ns in range(4):
                        c = half * 512 + ns * 128
                        nc.tensor.transpose(pvt[:, c:c + 128],
                                            vtb[:, c:c + 128], ident[:])
                    for ns in range(4):
                        kc = (2 * nbp + half) * 4 + ns
                        c0 = kc * 256
                        c = half * 512 + ns * 128
                        nc.vector.tensor_scalar_mul(
                            vw8[:, c0 + 64:c0 + 128],
                            pvt[:, c:c + 64], w_s[:, kc:kc + 1])
                        nc.vector.tensor_scalar_mul(
                            vw8[:, c0 + 192:c0 + 256],
                            pvt[:, c + 64:c + 128], w_s[:, kc:kc + 1])
                psq = psS.tile([128, 1024], FP, name="pst")
                proj_mm(psq, wq_b, ncol)
                # striped query layout: phys col s*512 + nb*64 + t holds
                # global query nb*512 + s*64 + t
                for ht in range(2):
                    rows = slice(64 * ht, 64 * ht + 64)
                    q_dst = qab_t[rows, ht * N:(ht + 1) * N].rearrange(
                        "p (s j t) -> p s j t", j=8, t=64)[:, :, 2 * nbp:2 * nbp + 2, :]
                    nc.scalar.activation(
                        q_dst,
                        psq[rows, :].rearrange("p (j s t) -> p s j t", j=2, t=64),
                        AF.Identity, bias=bq_s[rows, 0:1])
                if nbp < 3:  # prefetch next x pair (keeps first-MM deps tight)
                    load_pair(nbp + 1)

            # ------- prefetch: out-proj consts (DMA-idle window) -----------
            nc.sync.dma_start(wo_b[:].rearrange("p (cc o) -> p cc o", o=D),
                              wot.rearrange("(cc p) o -> p cc o", p=128))
            bost = p1s.tile([128, D], FP, name="bost")
            nc.sync.dma_start(bost[:], bov.unsqueeze(0).broadcast_to([128, D]))
            for j in range(4):
                xrt = p1s.tile([128, D], FP, name=f"xrt{j % 2}")
                (nc.sync if j % 2 == 0 else nc.scalar).dma_start(
                    xrt[:], xr[j * 128:(j + 1) * 128, :])
                nc.gpsimd.tensor_tensor(xb_s[j][:], xrt[:], bost[:], op=ALU.add)

            # ---------- phase 2: attention loop ----------------------------
            pb_q = []  # deferred PV inputs: (pso set, q5, pr, pb8 tile)

            def normalize(psoX, q5_):
                # ao = pso[64:128] * recip(pso[0]) -> fp8, then stage for A2A
                for h in range(2):
                    rc = p2n.tile([1, 512], FP, name=f"rc{h}")
                    nc.vector.reciprocal_approx_fast(rc[:], psoX[h][0:1, :])
                    bch = p2n.tile([128, 512], FP, name=f"bc{h}")
                    nc.gpsimd.partition_broadcast(bch[:], rc[:])
                    aot = aop.tile([128, 512], F8, name=f"ao{h}")
                    nc.vector.tensor_mul(aot[64:128, :], psoX[h][64:128, :],
                                         bch[64:128, :])
                    # stage phys block q5_ (stripe q5_ of every dest)
                    qq, bc = q5_ // 2, q5_ % 2
                    nc.sync.dma_start(
                        cc_in[qq][:].rearrange("(j g) c -> g j c", g=128)
                        [h * 64:(h + 1) * 64, :, bc * 64:(bc + 1) * 64],
                        aot[64:128, :].rearrange("p (j t) -> p j t", t=64))
                    if dbg is not None:
                        nc.scalar.dma_start(
                            dbg["d_ao"][h * 64:(h + 1) * 64,
                                        q5_ * 512:(q5_ + 1) * 512],
                            aot[64:128, :])
                if use_collective and q5_ in (1, 3, 5):
                    a2a(q5_ // 2)  # both blocks of the quarter now staged


            def emit_pv():
                psoX, q5_, pr_, pbt_ = pb_q.pop(0)
                pbr_ = pbt_[:].rearrange("p (kc h c) -> p kc h c", kc=2, h=2)
                for h in range(2):
                    nc.tensor.matmul(
                        psoX[h][:], vw8_r[:, pr_, :, 128 * h:128 * h + 128],
                        pbr_[:, :, h],
                        start=(pr_ == 0), stop=(pr_ == 15), perf_mode=DR)
                if pr_ == 15:  # q5_ complete: normalize while q5_+1 computes
                    normalize(psoX, q5_)

            for q5 in range(N // 512):
                qcol = slice(q5 * 512, (q5 + 1) * 512)
                pso = [psO.tile([128, 512], FP, name=f"pso{h}") for h in range(2)]
                for pr in range(16):
                    pb8 = p2s.tile([128, 2048], F8, name="pb8")
                    for i in range(2):
                        kc = 2 * pr + i
                        krng = slice(kc * 128, (kc + 1) * 128)
                        # one [128,1024] psum tile per kc: head0 S in cols
                        # 0:512, head1 in 512:1024 (zero-padded K=128)
                        pst = psS.tile([128, 1024], FP, name="pst")
                        nc.tensor.matmul(pst[:, 0:512], kb_t[:, krng],
                                         qab_t[:, qcol], start=True, stop=True)
                        nc.tensor.matmul(
                            pst[:, 512:1024], kb_t[:, krng],
                            qab_t[:, N + q5 * 512:N + (q5 + 1) * 512],
                            start=True, stop=True)
                        # whole-tile exp, strictly alternating engines so no
                        # two consecutive tiles hit the same engine (stalls)
                        if kc % 2 == 1:
                            dst = pb8[:].bitcast(I8)[:, i * 1024:(i + 1) * 1024]
                            nc.vector.tensor_scalar(dst, pst[:], BPRIME, 0.0,
                                                    op0=ALU.add, op1=ALU.max)
                        else:
                            nc.scalar.activation(
                                pb8[:, i * 1024:(i + 1) * 1024], pst[:],
                                AF.Exp, bias=cbias[:, 0:1], scale=cscale[:, 0:1])
                    pb_q.append((pso, q5, pr, pb8))
                    if len(pb_q) > 2:
                        emit_pv()  # deferred 2 pairs; crosses q5 boundaries
            while pb_q:
                emit_pv()  # prs 14, 15 of the last block (+ its normalize)

        # ---------------- phase 3: last A2A + out projection ---------------
        with tc.tile_pool(name="p3s" + sfx, bufs=16) as p3s, \
             tc.tile_pool(name="p3f" + sfx, bufs=4) as p3f, \
             tc.tile_pool(name="p3p", bufs=1, space="PSUM") as p3p:
            if use_collective:
                a2a(3)
            else:  # single-core timing-sim stand-in
                for i in range(NQ):
                    nc.sync.dma_start(cc_out[i][:], cc_in[i][:])
            # quarter qq = my output rows qq*128:(qq+1)*128 (after striping)
            psf = [p3p.tile([128, 512], FP, name=f"psf{t}") for t in range(8)]
            wo_r = wo_b[:].rearrange("p (pr two o) -> p pr two o", two=2, o=D)
            for qq in range(NQ):
                aocs = []
                for cp in range(4):
                    aoc = p3s.tile([128, 256], F8, name="aoc")
                    (nc.sync, nc.scalar, nc.gpsimd)[cp % 3].dma_start(
                        aoc[:].rearrange("p (two q) -> p two q", two=2),
                        cc_out[qq][cp * 256:(cp + 1) * 256, :].rearrange(
                            "(two p) q -> p two q", p=128))
                    aocs.append(aoc)
                for cp in range(4):
                    aoc_r = aocs[cp][:].rearrange("p (two q) -> p two q", two=2)
                    for ob in range(2):
                        nc.tensor.matmul(
                            psf[qq * 2 + ob][:], aoc_r,
                            wo_r[:, cp, :, ob * 512:(ob + 1) * 512],
                            start=(cp == 0), stop=(cp == 3), perf_mode=DR)
                for ob in range(2):
                    t = qq * 2 + ob
                    fo = p3f.tile([128, 512], FP, name="fo")
                    nc.vector.tensor_add(fo[:], psf[t][:],
                                         xb_s[qq][:, ob * 512:(ob + 1) * 512])
                    (nc.sync, nc.scalar, nc.gpsimd)[t % 3].dma_start(
                        out[qq * 128:(qq + 1) * 128,
                            ob * 512:(ob + 1) * 512], fo[:])

    if dbg is not None:
        for nm, t in (("d_k", kb_t), ("d_vw", vw8)):
            nc.sync.dma_start(dbg[nm], t[:])
        for i in range(NQ):
            nc.sync.dma_start(dbg["d_cc"][:, i * 128:(i + 1) * 128], cc_out[i][:])
    dram.release()
    per.release()
    cst.release()


def build_nc(reps=1, use_collective=True, debug=False):
    nc = bacc.Bacc("TRN2", target_bir_lowering=False, debug=False, num_devices=NCORE)
    xt = nc.dram_tensor("xt", [D, N], F8, kind="ExternalInput").ap()
    xr = nc.dram_tensor("xr", [NR, D], FP, kind="ExternalInput").ap()
    wqt = nc.dram_tensor("wqt", [D, CB], F8, kind="ExternalInput").ap()
    wkt = nc.dram_tensor("wkt", [D, CB], F8, kind="ExternalInput").ap()
    wvt = nc.dram_tensor("wvt", [D, CB], F8, kind="ExternalInput").ap()
    wot = nc.dram_tensor("wot", [D, D], F8, kind="ExternalInput").ap()
    bqv = nc.dram_tensor("bqv", [CB], FP, kind="ExternalInput").ap()
    bkv = nc.dram_tensor("bkv", [CB], FP, kind="ExternalInput").ap()
    bvv = nc.dram_tensor("bvv", [CB], FP, kind="ExternalInput").ap()
    bov = nc.dram_tensor("bov", [D], FP, kind="ExternalInput").ap()
    # h pre-rearranged on host to [128, KCH]: hv[p, c] = h[c*128 + p]
    hv = nc.dram_tensor("hv", [128, KCH], FP, kind="ExternalInput").ap()
    out = nc.dram_tensor("out", [NR, D], FP, kind="ExternalOutput").ap()
    dbg = None
    if debug:
        dbg = {
            "d_q": nc.dram_tensor("d_q", [128, N], BF, kind="ExternalOutput").ap(),
            "d_k": nc.dram_tensor("d_k", [128, N], BF, kind="ExternalOutput").ap(),
            "d_vw": nc.dram_tensor("d_vw", [128, KCH * 256], F8, kind="ExternalOutput").ap(),
            "d_ao": nc.dram_tensor("d_ao", [128, N], F8, kind="ExternalOutput").ap(),
            "d_cc": nc.dram_tensor("d_cc", [NCORE * 128, NR], F8, kind="ExternalOutput").ap(),
        }
    with tile.TileContext(nc) as tc:
        _body(nc, tc, reps, xt, xr, wqt, wkt, wvt, wot,
              bqv, bkv, bvv, bov, hv, out, use_collective=use_collective, dbg=dbg)
    nc.compile()
    return nc


_NC_CACHE = {}


def get_nc(reps=1):
    if reps not in _NC_CACHE:
        _NC_CACHE[reps] = build_nc(reps)
    return _NC_CACHE[reps]


def make_in_maps(inputs):
    x = np.ascontiguousarray(np.asarray(inputs["x"], dtype=np.float32))
    h = np.ascontiguousarray(np.asarray(inputs["h"], dtype=np.float32))
    Wq = np.asarray(inputs["Wq"], dtype=np.float32)
    bq = np.asarray(inputs["bq"], dtype=np.float32)
    Wk = np.asarray(inputs["Wk"], dtype=np.float32)
    bk = np.asarray(inputs["bk"], dtype=np.float32)
    Wv = np.asarray(inputs["Wv"], dtype=np.float32)
    bv = np.asarray(inputs["bv"], dtype=np.float32)
    Wo = np.asarray(inputs["Wo"], dtype=np.float32)
    bo = np.ascontiguousarray(np.asarray(inputs["bo"], dtype=np.float32))
    import ml_dtypes
    f8 = ml_dtypes.float8_e4m3
    xt = np.ascontiguousarray(x.T.astype(f8))
    wot = np.ascontiguousarray(Wo.T.astype(f8))
    qscale = np.float32(0.125 * M8)  # 1/sqrt(dh) * Schraudolph scale
    in_maps = []
    for i in range(NCORE):
        cs = slice(i * CB, (i + 1) * CB)
        in_maps.append({
            "xt": xt,
            "xr": np.ascontiguousarray(x[i * NR:(i + 1) * NR, :]),
            "wqt": np.ascontiguousarray((Wq[cs, :] * qscale).T.astype(f8)),
            "wkt": np.ascontiguousarray(Wk[cs, :].T.astype(f8)),
            "wvt": np.ascontiguousarray(Wv[cs, :].T.astype(f8)),
            "wot": wot,
            "bqv": np.ascontiguousarray(bq[cs] * qscale),
            "bkv": np.ascontiguousarray(bk[cs]),
            "bvv": np.ascontiguousarray(bv[cs]),
            "bov": bo,
            "hv": np.ascontiguousarray(h.reshape(KCH, 128).T),
        })
    return in_maps


def kernel(**inputs):
    nc = get_nc(1)
    in_maps = make_in_maps(inputs)
    res = run_bass_kernel_spmd(nc, in_maps, core_ids=list(range(NCORE)))
    return np.concatenate([res.results[i]["out"] for i in range(NCORE)], axis=0)


# revision 45
# speedup vs baseline: 1.0189x; 1.0042x over previous
"""Trainium2 Bass kernel for nn_MILPAttention (dense multi-head attention with
per-key additive bias), tensor-parallel over heads across 8 NeuronCores.

Self-contained: hardcodes shapes N=4096, D=1024, H=16, GAMMA=1.0.

Math (reference):
    q = x @ Wq.T + bq ; k = x @ Wk.T + bk ; v = x @ Wv.T + bv     (per head, dh=64)
    logits = (q @ k.T) / 8 - h[key]
    attn = softmax(logits, keys)
    out = (attn @ v) @ Wo.T + bo + x

Per-core strategy (core i owns heads 2i, 2i+1 = columns 128i:128(i+1)):
  - Projections run as fp8 DoubleRow matmuls over resident x^T, evacuated
    at N=1024. q is computed into two half-zeroed tiles qa (head0 in rows
    0:64) / qb (head1 in rows 64:128) so every S matmul runs with a full
    K=128 contraction; q is pre-scaled by m8 = 8/ln2 (Schraudolph prep).
  - S^T per key chunk: head0 into cols 0:512, head1 into 512:1024 of one
    [128,1024] PSUM tile, consumed WHOLE by one exp engine at N=1024.
    Scalar tiles: true exp (ACT Exp -> fp8e4). Vector tiles: Schraudolph
    tensor_scalar add+max -> int8 bits = fp8e4 of exp(l-c). An 18/14 kc
    split balances the engines; ScalarE+VectorE PSUM-evacuation throughput
    and the PE (at its ~2.0 GHz sustained clock) are jointly the roofline.
    The global shift c cancels in the softmax ratio.
  - P@V as fp8 DoubleRow matmuls, deferred two pairs so the in-order PE
    queue never waits on the exp engines. V transposed via PE, scaled by
    w = exp(-h) (folds the per-key bias), stored fp8 with w as a 65th
    stationary column so PV also yields the softmax denominator.
  - Normalize reads the PV psum directly (no snap copy): DVE reciprocal of
    the denominator row, gpsimd partition-broadcast, DVE multiply -> fp8.
  - 4-way split AllToAll (fp8) switches head-sharding -> sequence-sharding
    (quarters issued at q-blocks 2/4/6/7; ~16us latency each, only the
    last is exposed); out projection fp8 DoubleRow per quarter, bias +
    residual in fp32.
"""
import numpy as np

import concourse.bass as bass
import concourse.mybir as mybir
import concourse.tile as tile
from concourse import bacc
from concourse.bass_utils import run_bass_kernel_spmd
from concourse.masks import make_identity

N, D, H = 4096, 1024, 16
NCORE = 8
CB = D // NCORE          # 128 columns (2 heads) per core
NR = N // NCORE          # 512 output rows per core
DH = D // H              # 64
KCH = N // 128           # 32 key chunks
NB = N // 512            # 8 n-blocks
FP = mybir.dt.float32
BF = mybir.dt.bfloat16
F8 = mybir.dt.float8e4
I8 = mybir.dt.int8
AF = mybir.ActivationFunctionType
ALU = mybir.AluOpType
DR = mybir.MatmulPerfMode.DoubleRow

M8 = 8.0 / np.log(2.0)      # 11.5416 Schraudolph scale, folded into Wq on host
CSHIFT = 4.8                # global logit shift: P' = exp(l - c), cancels in softmax
CORR = -0.47                # Schraudolph correction (round-to-nearest tuned)
BPRIME = 56.0 + CORR - M8 * CSHIFT   # add constant (psum already holds m8*l)

NQ = 4                       # A2A quarters, 2 phys q-blocks (128 rows) each


def _body(nc, tc, reps, xt, xr, wqt, wkt, wvt, wot, bqv, bkv, bvv, bov, hv, out,
          use_collective=True, dbg=None):
    cst = tc.alloc_tile_pool(name="cst", bufs=1)
    per = tc.alloc_tile_pool(name="per", bufs=1)
    dram = tc.alloc_tile_pool(name="dram", bufs=1, space="DRAM")

    ident = cst.tile([128, 128], BF)
    make_identity(nc, ident[:])

    # persistent sbuf
    wq_b = per.tile([128, D], F8)        # [d-in-chunk, dc*128 + c]
    wk_b = per.tile([128, D], F8)
    wv_b = per.tile([128, D], F8)
    wo_b = per.tile([128, 8 * D], F8)    # [c-in-chunk, cc*1024 + o]
    # concatenated q: cols 0:N = head0 q (rows 0:64 real, 64:128 zero),
    # cols N:2N = head1 q (rows 64:128 real, 0:64 zero). One [128,2,512]
    # strided moving AP then computes BOTH heads' S in a single matmul.
    qab_t = per.tile([128, 2 * N], BF)
    kb_t = per.tile([128, N], BF)        # kT: rows = 2 heads x 64 dims
    vw8 = per.tile([128, KCH * 256], F8)  # per kc, per head: [w|pad63|v64]
    bq_s = per.tile([128, 1], FP)
    bk_s = per.tile([128, 1], FP)
    bv_s = per.tile([128, 1], FP)
    cbias = per.tile([128, 1], FP)       # -CSHIFT for the scalar exp
    cscale = per.tile([128, 1], FP)      # 1/M8 for the scalar exp
    w_s = per.tile([128, KCH], FP)       # exp(-h), [key-in-chunk, chunk]
    xb_s = [per.tile([128, D], FP, name=f"xb{j}") for j in range(4)]  # x rows + bo
    x_all = per.tile([128, 8 * N], F8)   # x^T resident, col = dc*N + n
    # persistent aoc tiles for quarters 0-2: prefetched at the end of phase 2
    # (their collectives completed mid-loop) so phase-3 outproj starts cold-free
    aoc_p = [per.tile([128, 256], F8, name=f"aocp{qq}_{cp}")
             for qq in range(3) for cp in range(4)]

    cc_in = [dram.tile([NCORE * 128, 128], F8, name=f"ccin{i}") for i in range(NQ)]
    cc_out = [dram.tile([NCORE * 128, 128], F8, name=f"ccout{i}") for i in range(NQ)]

    def a2a(qq):
        nc.gpsimd.collective_compute(
            "AllToAll", mybir.AluOpType.bypass,
            replica_groups=[list(range(NCORE))],
            ins=[cc_in[qq][:].opt()], outs=[cc_out[qq][:].opt()])

    for rep in range(reps):
        sfx = f"_{rep}"
        # ---------------- phase 0: constants ------------------------------
        hst = per.tile([128, KCH], FP, name="hst")
        nc.sync.dma_start(hst[:], hv)
        nc.scalar.activation(w_s[:], hst[:], AF.Exp, scale=-1.0)
        nc.sync.dma_start(bq_s[:], bqv.unsqueeze(1))
        nc.scalar.dma_start(bk_s[:], bkv.unsqueeze(1))
        nc.scalar.dma_start(bv_s[:], bvv.unsqueeze(1))
        nc.gpsimd.memset(cbias[:], -CSHIFT)
        nc.gpsimd.memset(cscale[:], 1.0 / M8)
        nc.vector.memset(qab_t[64:128, 0:N], 0.0)
        nc.gpsimd.memset(qab_t[0:64, N:2 * N], 0.0)
        for wi, (wsrc, wdst) in enumerate(
                ((wqt, wq_b), (wkt, wk_b), (wvt, wv_b))):
            eng = (nc.sync, nc.scalar, nc.scalar)[wi]
            eng.dma_start(wdst[:].rearrange("p (dc c) -> p dc c", c=CB),
                          wsrc.rearrange("(dc p) c -> p dc c", p=128))
        # w columns of vw8 (first col of each head block) <- w_s[:, kc]
        for h in range(2):
            nc.vector.tensor_copy(
                vw8[:].rearrange("p (kc c) -> p kc c", c=256)[:, :, 128 * h],
                w_s[:])

        # ------ phases 1+2 share one scope (no cross-phase PE barrier) -----
        vw8_r = vw8[:].rearrange("p (pr two c) -> p pr two c", two=2, c=256)
        with tc.tile_pool(name="p1s" + sfx, bufs=4) as p1s, \
             tc.tile_pool(name="p2s" + sfx, bufs=5) as p2s, \
             tc.tile_pool(name="aop" + sfx, bufs=4) as aop, \
             tc.tile_pool(name="p2n", bufs=2) as p2n, \
             tc.tile_pool(name="psS", bufs=3, space="PSUM") as psS, \
             tc.tile_pool(name="psO", bufs=1, space="PSUM") as psO:
            # ---------- phase 1: x preload + projections (N=1024 blocks) ---
            def load_pair(nbp):
                for nb in (2 * nbp, 2 * nbp + 1):
                    for dc in range(8):
                        (nc.sync, nc.scalar, nc.gpsimd)[dc % 3].dma_start(
                            x_all[:, dc * N + nb * 512:dc * N + (nb + 1) * 512],
                            xt[dc * 128:(dc + 1) * 128, nb * 512:(nb + 1) * 512])

            def proj_mm(ps, w_b, ncol1024):
                # DR moving operand caps at 2x512 elements: two 512-col groups
                for g in range(2):
                    ncol = slice(ncol1024.start + g * 512,
                                 ncol1024.start + (g + 1) * 512)
                    for dp in range(4):
                        nc.tensor.matmul(
                            ps[:, g * 512:(g + 1) * 512],
                            w_b[:].rearrange("p (dp two c) -> p dp two c",
                                             two=2, c=CB)[:, dp],
                            x_all[:].rearrange("p (dp two n) -> p dp two n",
                                               two=2, n=N)[:, dp, :, ncol],
                            start=(dp == 0), stop=(dp == 3), perf_mode=DR)

            load_pair(0)
            for nbp in range(4):
                ncol = slice(nbp * 1024, (nbp + 1) * 1024)
                psk = psS.tile([128, 1024], FP, name="pst")
                proj_mm(psk, wk_b, ncol)
                nc.scalar.activation(kb_t[:, ncol], psk[:], AF.Identity,
                                     bias=bk_s[:, 0:1])
                psv = psS.tile([128, 1024], FP, name="pst")
                proj_mm(psv, wv_b, ncol)
                vtb = p1s.tile([128, 1024], BF, name="vtb")
                nc.vector.tensor_scalar_add(vtb[:], psv[:], bv_s[:, 0:1])
                pvt_f = psS.tile([128, 1024], FP, name="pst")
                pvt = pvt_f[:].bitcast(BF)  # [128, 2048] bf16 view
                for half in range(2):  # nb = 2*nbp + half
                    for ns in range(4):
                        c = half * 512 + ns * 128
                        nc.tensor.transpose(pvt[:, c:c + 128],
                                            vtb[:, c:c + 128], ident[:])
                    for ns in range(4):
                        kc = (2 * nbp + half) * 4 + ns
                        c0 = kc * 256
                        c = half * 512 + ns * 128
                        nc.vector.tensor_scalar_mul(
                            vw8[:, c0 + 64:c0 + 128],
                            pvt[:, c:c + 64], w_s[:, kc:kc + 1])
                        nc.vector.tensor_scalar_mul(
                            vw8[:, c0 + 192:c0 + 256],
                            pvt[:, c + 64:c + 128], w_s[:, kc:kc + 1])
                psq = psS.tile([128, 1024], FP, name="pst")
                proj_mm(psq, wq_b, ncol)
                # striped query layout: phys col s*512 + nb*64 + t holds
                # global query nb*512 + s*64 + t
                for ht in range(2):
                    rows = slice(64 * ht, 64 * ht + 64)
                    q_dst = qab_t[rows, ht * N:(ht + 1) * N].rearrange(
                        "p (s j t) -> p s j t", j=8, t=64)[:, :, 2 * nbp:2 * nbp + 2, :]
                    nc.scalar.activation(
                        q_dst,
                        psq[rows, :].rearrange("p (j s t) -> p s j t", j=2, t=64),
                        AF.Identity, bias=bq_s[rows, 0:1])
                if nbp < 3:  # prefetch next x pair (keeps first-MM deps tight)
                    load_pair(nbp + 1)

            # ------- prefetch: out-proj consts (DMA-idle window) -----------
            nc.sync.dma_start(wo_b[:].rearrange("p (cc o) -> p cc o", o=D),
                              wot.rearrange("(cc p) o -> p cc o", p=128))
            bost = p1s.tile([128, D], FP, name="bost")
            nc.sync.dma_start(bost[:], bov.unsqueeze(0).broadcast_to([128, D]))
            for j in range(4):
                xrt = p1s.tile([128, D], FP, name=f"xrt{j % 2}")
                (nc.sync if j % 2 == 0 else nc.scalar).dma_start(
                    xrt[:], xr[j * 128:(j + 1) * 128, :])
                nc.gpsimd.tensor_tensor(xb_s[j][:], xrt[:], bost[:], op=ALU.add)

            # ---------- phase 2: attention loop ----------------------------
            pb_q = []  # deferred PV inputs: (pso set, q5, pr, pb8 tile)

            def normalize(psoX, q5_):
                # ao = pso[64:128] * recip(pso[0]) -> fp8, then stage for A2A
                for h in range(2):
                    rc = p2n.tile([1, 512], FP, name=f"rc{h}")
                    nc.vector.reciprocal_approx_fast(rc[:], psoX[h][0:1, :])
                    bch = p2n.tile([128, 512], FP, name=f"bc{h}")
                    nc.gpsimd.partition_broadcast(bch[:], rc[:])
                    aot = aop.tile([128, 512], F8, name=f"ao{h}")
                    nc.vector.tensor_mul(aot[64:128, :], psoX[h][64:128, :],
                                         bch[64:128, :])
                    # stage phys block q5_ (stripe q5_ of every dest)
                    qq, bc = q5_ // 2, q5_ % 2
                    nc.sync.dma_start(
                        cc_in[qq][:].rearrange("(j g) c -> g j c", g=128)
                        [h * 64:(h + 1) * 64, :, bc * 64:(bc + 1) * 64],
                        aot[64:128, :].rearrange("p (j t) -> p j t", t=64))
                    if dbg is not None:
                        nc.scalar.dma_start(
                            dbg["d_ao"][h * 64:(h + 1) * 64,
                                        q5_ * 512:(q5_ + 1) * 512],
                            aot[64:128, :])
                if use_collective and q5_ in (1, 3, 5):
                    a2a(q5_ // 2)  # both blocks of the quarter now staged


            def emit_pv():
                psoX, q5_, pr_, pbt_ = pb_q.pop(0)
                pbr_ = pbt_[:].rearrange("p (kc h c) -> p kc h c", kc=2, h=2)
                for h in range(2):
                    nc.tensor.matmul(
                        psoX[h][:], vw8_r[:, pr_, :, 128 * h:128 * h + 128],
                        pbr_[:, :, h],
                        start=(pr_ == 0), stop=(pr_ == 15), perf_mode=DR)
                if pr_ == 15:  # q5_ complete: normalize while q5_+1 computes
                    normalize(psoX, q5_)

            for q5 in range(N // 512):
                qcol = slice(q5 * 512, (q5 + 1) * 512)
                pso = [psO.tile([128, 512], FP, name=f"pso{h}") for h in range(2)]
                for pr in range(16):
                    pb8 = p2s.tile([128, 2048], F8, name="pb8")
                    for i in range(2):
                        kc = 2 * pr + i
                        krng = slice(kc * 128, (kc + 1) * 128)
                        # one [128,1024] psum tile per kc: head0 S in cols
                        # 0:512, head1 in 512:1024 (zero-padded K=128)
                        pst = psS.tile([128, 1024], FP, name="pst")
                        nc.tensor.matmul(pst[:, 0:512], kb_t[:, krng],
                                         qab_t[:, qcol], start=True, stop=True)
                        nc.tensor.matmul(
                            pst[:, 512:1024], kb_t[:, krng],
                            qab_t[:, N + q5 * 512:N + (q5 + 1) * 512],
                            start=True, stop=True)
                        # whole-tile exp, strictly alternating engines so no
                        # two consecutive tiles hit the same engine (stalls)
                        if kc % 2 == 1:
                            dst = pb8[:].bitcast(I8)[:, i * 1024:(i + 1) * 1024]
                            nc.vector.tensor_scalar(dst, pst[:], BPRIME, 0.0,
                                                    op0=ALU.add, op1=ALU.max)
                        else:
                            nc.scalar.activation(
                                pb8[:, i * 1024:(i + 1) * 1024], pst[:],
                                AF.Exp, bias=cbias[:, 0:1], scale=cscale[:, 0:1])
                    pb_q.append((pso, q5, pr, pb8))
                    if len(pb_q) > 2:
                        emit_pv()  # deferred 2 pairs; crosses q5 boundaries
            while pb_q:
                emit_pv()  # prs 14, 15 of the last block (+ its normalize)
            for qq in range(3 if use_collective else 0):
                # prefetch completed quarters' A2A results
                for cp in range(4):
                    (nc.sync, nc.scalar)[cp % 2].dma_start(
                        aoc_p[qq * 4 + cp][:].rearrange(
                            "p (two q) -> p two q", two=2),
                        cc_out[qq][cp * 256:(cp + 1) * 256, :].rearrange(
                            "(two p) q -> p two q", p=128))

        # ---------------- phase 3: last A2A + out projection ---------------
        with tc.tile_pool(name="p3s" + sfx, bufs=16) as p3s, \
             tc.tile_pool(name="p3f" + sfx, bufs=4) as p3f, \
             tc.tile_pool(name="p3p", bufs=1, space="PSUM") as p3p:
            if use_collective:
                a2a(3)
            else:  # single-core timing-sim stand-in
                for i in range(NQ):
                    nc.sync.dma_start(cc_out[i][:], cc_in[i][:])
            # quarter qq = my output rows qq*128:(qq+1)*128 (after striping)
            psf = [p3p.tile([128, 512], FP, name=f"psf{t}") for t in range(8)]
            wo_r = wo_b[:].rearrange("p (pr two o) -> p pr two o", two=2, o=D)
            for qq in range(NQ):
                if use_collective and qq < 3:
                    aocs = aoc_p[qq * 4:(qq + 1) * 4]  # prefetched in phase 2
                else:
                    aocs = []
                    for cp in range(4):
                        aoc = p3s.tile([128, 256], F8, name="aoc")
                        (nc.sync, nc.scalar, nc.gpsimd)[cp % 3].dma_start(
                            aoc[:].rearrange("p (two q) -> p two q", two=2),
                            cc_out[qq][cp * 256:(cp + 1) * 256, :].rearrange(
                                "(two p) q -> p two q", p=128))
                        aocs.append(aoc)
                for cp in range(4):
                    aoc_r = aocs[cp][:].rearrange("p (two q) -> p two q", two=2)
                    for ob in range(2):
                        nc.tensor.matmul(
                            psf[qq * 2 + ob][:], aoc_r,
                            wo_r[:, cp, :, ob * 512:(ob + 1) * 512],
                            start=(cp == 0), stop=(cp == 3), perf_mode=DR)
                for ob in range(2):
                    t = qq * 2 + ob
                    fo = p3f.tile([128, 512], FP, name="fo")
                    nc.vector.tensor_add(fo[:], psf[t][:],
                                         xb_s[qq][:, ob * 512:(ob + 1) * 512])
                    (nc.sync, nc.scalar)[t % 2].dma_start(
                        out[qq * 128:(qq + 1) * 128,
                            ob * 512:(ob + 1) * 512], fo[:])

    if dbg is not None:
        for nm, t in (("d_k", kb_t), ("d_vw", vw8)):
            nc.sync.dma_start(dbg[nm], t[:])
        for i in range(NQ):
            nc.sync.dma_start(dbg["d_cc"][:, i * 128:(i + 1) * 128], cc_out[i][:])
    dram.release()
    per.release()
    cst.release()


def build_nc(reps=1, use_collective=True, debug=False):
    nc = bacc.Bacc("TRN2", target_bir_lowering=False, debug=False, num_devices=NCORE)
    xt = nc.dram_tensor("xt", [D, N], F8, kind="ExternalInput").ap()
    xr = nc.dram_tensor("xr", [NR, D], FP, kind="ExternalInput").ap()
    wqt = nc.dram_tensor("wqt", [D, CB], F8, kind="ExternalInput").ap()
    wkt = nc.dram_tensor("wkt", [D, CB], F8, kind="ExternalInput").ap()
    wvt = nc.dram_tensor("wvt", [D, CB], F8, kind="ExternalInput").ap()
    wot = nc.dram_tensor("wot", [D, D], F8, kind="ExternalInput").ap()
    bqv = nc.dram_tensor("bqv", [CB], FP, kind="ExternalInput").ap()
    bkv = nc.dram_tensor("bkv", [CB], FP, kind="ExternalInput").ap()
    bvv = nc.dram_tensor("bvv", [CB], FP, kind="ExternalInput").ap()
    bov = nc.dram_tensor("bov", [D], FP, kind="ExternalInput").ap()
    # h pre-rearranged on host to [128, KCH]: hv[p, c] = h[c*128 + p]
    hv = nc.dram_tensor("hv", [128, KCH], FP, kind="ExternalInput").ap()
    out = nc.dram_tensor("out", [NR, D], FP, kind="ExternalOutput").ap()
    dbg = None
    if debug:
        dbg = {
            "d_q": nc.dram_tensor("d_q", [128, N], BF, kind="ExternalOutput").ap(),
            "d_k": nc.dram_tensor("d_k", [128, N], BF, kind="ExternalOutput").ap(),
            "d_vw": nc.dram_tensor("d_vw", [128, KCH * 256], F8, kind="ExternalOutput").ap(),
            "d_ao": nc.dram_tensor("d_ao", [128, N], F8, kind="ExternalOutput").ap(),
            "d_cc": nc.dram_tensor("d_cc", [NCORE * 128, NR], F8, kind="ExternalOutput").ap(),
        }
    with tile.TileContext(nc) as tc:
        _body(nc, tc, reps, xt, xr, wqt, wkt, wvt, wot,
              bqv, bkv, bvv, bov, hv, out, use_collective=use_collective, dbg=dbg)
    nc.compile()
    return nc


_NC_CACHE = {}


def get_nc(reps=1):
    if reps not in _NC_CACHE:
        _NC_CACHE[reps] = build_nc(reps)
    return _NC_CACHE[reps]


def make_in_maps(inputs):
    x = np.ascontiguousarray(np.asarray(inputs["x"], dtype=np.float32))
    h = np.ascontiguousarray(np.asarray(inputs["h"], dtype=np.float32))
    Wq = np.asarray(inputs["Wq"], dtype=np.float32)
    bq = np.asarray(inputs["bq"], dtype=np.float32)
    Wk = np.asarray(inputs["Wk"], dtype=np.float32)
    bk = np.asarray(inputs["bk"], dtype=np.float32)
    Wv = np.asarray(inputs["Wv"], dtype=np.float32)
    bv = np.asarray(inputs["bv"], dtype=np.float32)
    Wo = np.asarray(inputs["Wo"], dtype=np.float32)
    bo = np.ascontiguousarray(np.asarray(inputs["bo"], dtype=np.float32))
    import ml_dtypes
    f8 = ml_dtypes.float8_e4m3
    xt = np.ascontiguousarray(x.T.astype(f8))
    wot = np.ascontiguousarray(Wo.T.astype(f8))
    qscale = np.float32(0.125 * M8)  # 1/sqrt(dh) * Schraudolph scale
    in_maps = []
    for i in range(NCORE):
        cs = slice(i * CB, (i + 1) * CB)
        in_maps.append({
            "xt": xt,
            "xr": np.ascontiguousarray(x[i * NR:(i + 1) * NR, :]),
            "wqt": np.ascontiguousarray((Wq[cs, :] * qscale).T.astype(f8)),
            "wkt": np.ascontiguousarray(Wk[cs, :].T.astype(f8)),
            "wvt": np.ascontiguousarray(Wv[cs, :].T.astype(f8)),
            "wot": wot,
            "bqv": np.ascontiguousarray(bq[cs] * qscale),
            "bkv": np.ascontiguousarray(bk[cs]),
            "bvv": np.ascontiguousarray(bv[cs]),
            "bov": bo,
            "hv": np.ascontiguousarray(h.reshape(KCH, 128).T),
        })
    return in_maps


def kernel(**inputs):
    nc = get_nc(1)
    in_maps = make_in_maps(inputs)
    res = run_bass_kernel_spmd(nc, in_maps, core_ids=list(range(NCORE)))
    return np.concatenate([res.results[i]["out"] for i in range(NCORE)], axis=0)
